# revision 56
# baseline (speedup 1.0000x reference)
"""nn_MultiHeadAttention_59253368815813 on 8 TRN2 NeuronCores.

The reference module is bug-faithful to its original nn.Module in two ways
that together collapse the computation:

  1. ``o = jnp.einsum('bhtl,bthd->bhtd', A, v)`` indexes ``v`` by the QUERY
     position ``t``, not the key position ``l``. ``l`` therefore only sums
     over the softmax weights, which sum to exactly 1 per row:
     ``o[b,h,t,d] == v[b,t,h,d]``. Q, K, the mask and the softmax never
     influence the output.
  2. ``o.reshape(b, T, d)`` with no transpose scrambles (head, token): the
     reshaped activation row r = 128*h + s (s = t//16) takes column block
     m = t%16, i.e. Vscr[b][128h+s, 64m+dk] = (x@Wv)[b, 16s+m, 64h+dk].

So the exact computation is  out = Vscr @ Wo.T  with Vscr the scrambled
x@Wv.  Sharding (4 head-groups x 2 s-groups = 8 cores): core (j, g) owns
heads {4j..4j+3} (Wv cols [256j, 256j+256)) and s in [64g, 64g+64)
(tokens [1024g, 1024g+1024) per batch). Each core produces out rows
{128h + s : owned h, s} for both batches; the host reassembles. No
cross-core reduction.

All operands travel as bf16 (rel-err budget 2e-2; bf16 end-to-end lands
~3.4e-3) which both halves DMA and doubles PE rate vs fp32r (fp32r
measures ~466ns per 512-row matmul on HW; bf16 ~216ns).

Hardware lessons baked into the schedule (from perfetto/NTFF traces):
  - All DMA queues share the chip's 16 DMA engines (~350 GB/s per core);
    inter-ring arbitration is erratic (one ring can starve another 3:1),
    but a single ring processes transfers serially in trigger order. So
    ALL inputs ride the SP ring in exact consumption order:
    wv, x[Ng0a], x[Ng0b], x[Ng1], x[Ng2], x[Ng3], woT(k0-3), woT(k4-7).
  - The PE drops out of its 2.4 GHz p-state after any idle gap and takes
    ~3us of continuous work to return (matmuls run ~2x slower meanwhile).
    Dependency-free N=64 matmuls (53ns each) into dead psum bridge the
    DMA startup latency and pad expected stream-jitter points.
  - Matmul issue rate is column-limited (N=512 -> 216ns, N=256 -> 109ns,
    N=64 -> 53ns), so small matmuls cost nothing extra per column.

Per-core on-chip schedule, with token columns host-permuted to
u = m*128 + b*64 + s_lo so every shuffle copy is a contiguous block:
  vT[c, u] = Wv_slice^T @ xT           (4 N-groups of 512 tokens, 8 K)
  shuffle: VscrT[64(m%2)+dk, (b,q,hl,s)] = vT[64hl+dk, (m,b,s)]
           64 copies of [64p x 128] alternating DVE/Act (GpSimd cannot
           read PSUM), pipelined behind the v-proj N-groups
  out[64hl+s, n] = sum_k VscrT_k^T @ WoT_k   (8 accumulating matmuls per
           [128, 512] psum tile; 8 tiles in 2 waves of 4 banks, with the
           final tile's evacuation split across both copy engines and
           both DMA rings to shorten the tail)
"""

import sys
import types

import numpy as np

_TRN_REPO = "/opt/trn_rl_repo"
if _TRN_REPO not in sys.path:
    sys.path.insert(0, _TRN_REPO)


def _install_ntff_shim():
    """antenv.axon_hooks is absent in this container; provide it so
    BASS_TRACE=1 profiling works. No-op if the real module exists."""
    try:
        import antenv  # noqa: F401
    except ImportError:
        return
    if "antenv.axon_hooks" in sys.modules:
        return
    try:
        import antenv.axon_hooks  # noqa: F401
        return
    except ImportError:
        pass
    m = types.ModuleType("antenv.axon_hooks")
    m._hook = None
    m.set_axon_ntff_profile_hook = lambda h: setattr(m, "_hook", h)
    m.get_axon_ntff_profile_hook = lambda: m._hook
    sys.modules["antenv.axon_hooks"] = m
    try:
        from trn_agent_boot.trn_boot import _ntff_profile_via_ctypes

        hook = _ntff_profile_via_ctypes("/opt/axon/libaxon_pjrt.so")
        if hook is not None:
            m.set_axon_ntff_profile_hook(hook)
    except Exception:
        pass


_install_ntff_shim()

import ml_dtypes  # noqa: E402

import concourse.mybir as mybir  # noqa: E402
import concourse.tile as tile  # noqa: E402
from concourse import bacc  # noqa: E402
from concourse.bass_utils import run_bass_kernel_spmd  # noqa: E402

F32 = mybir.dt.float32
BF16 = mybir.dt.bfloat16
BF16_NP = np.dtype(ml_dtypes.bfloat16)

B = 2
T = 2048
D = 1024
NCORES = 8
HG = 4              # head groups (4 heads each)
SG = 2              # s groups (64 s-values each)
TOK = B * T // SG   # token columns per core = 2048
NWARM = 90          # N=64 PE warm-up matmuls to bridge DMA startup

_CACHED = None
LAST_RESULTS = None


def _build_module():
    nc = bacc.Bacc("TRN2", target_bir_lowering=False, debug=False,
                   num_devices=NCORES)

    xT_d = nc.dram_tensor("xT", [D, TOK], BF16, kind="ExternalInput").ap()
    wv_d = nc.dram_tensor("wv", [128, 8, 2, 128], BF16,
                          kind="ExternalInput").ap()
    wo_d = nc.dram_tensor("woT", [128, 8, D], BF16,
                          kind="ExternalInput").ap()
    out_d = nc.dram_tensor("out", [B, 2, 128, D], BF16,
                           kind="ExternalOutput").ap()

    with tile.TileContext(nc) as tc:
        _emit(nc, tc, xT_d, wv_d, wo_d, out_d)
    nc.compile()
    return nc


def _emit(nc, tc, xT_d, wv_d, wo_d, out_d):
    from contextlib import ExitStack

    ctx = ExitStack()
    with ctx:
        wpool = ctx.enter_context(tc.tile_pool(name="w", bufs=1))
        xtp = ctx.enter_context(tc.tile_pool(name="xt", bufs=4))
        vsp = ctx.enter_context(tc.tile_pool(name="vscr", bufs=8))
        outp = ctx.enter_context(tc.tile_pool(name="outsb", bufs=4))
        wmp = ctx.enter_context(tc.tile_pool(name="warm", bufs=2))
        ps_v = ctx.enter_context(tc.tile_pool(name="ps_v", bufs=4,
                                              space="PSUM"))
        ps_o = ctx.enter_context(tc.tile_pool(name="ps_o", bufs=4,
                                              space="PSUM"))

        # --- PE warm-up: keep the tensor engine busy (and ramping to its
        # top p-state) while the first DMAs are in flight. Warm matmuls
        # target the first real v-proj psum tile; its start=True reset
        # discards them, and the tile has real readers (the shuffle).
        wa = wmp.tile([128, 128], BF16, tag="wa")
        wb = wmp.tile([128, 512], BF16, tag="wb")
        nc.gpsimd.memset(wa[:], 0)
        nc.gpsimd.memset(wb[:], 0)
        psv0 = [ps_v.tile([128, 512], F32, tag="proj",
                          name=f"psv0_{ct}") for ct in range(2)]
        for i in range(NWARM):
            nc.tensor.matmul(psv0[0][:, :64], wa[:], wb[:, :64],
                             start=True, stop=True, skip_group_check=True)

        # --- input DMAs. Queue arbitration between rings is erratic (one
        # ring can starve another 3:1), but a single ring processes its
        # transfers serially in trigger order at the full ~350 GB/s. So ALL
        # inputs ride the SP ring in exact consumption order; outputs ride
        # the gpsimd SWDGE ring (idle engine, small late transfers); the
        # ACT ring carries nothing so Act is free to copy.
        wv_sb = wpool.tile([128, 8, 2, 128], BF16, tag="wv")
        nc.sync.dma_start(wv_sb[:], wv_d)
        xt0 = []
        for h in range(2):
            t = xtp.tile([128, 8, 256], BF16, tag="xt0", name=f"xt0{h}")
            nc.sync.dma_start(
                t[:], xT_d[:, h * 256:(h + 1) * 256]
                .rearrange("(ko ki) u -> ki ko u", ki=128))
            xt0.append(t)

        def wv_ap(k, ct):
            return wv_sb[:, k, ct, :]
        xt = [None]
        for a in (1, 2, 3):
            t = xtp.tile([128, 8, 512], BF16, tag="xt", name=f"xt{a}")
            nc.sync.dma_start(
                t[:], xT_d[:, a * 512:(a + 1) * 512]
                .rearrange("(ko ki) u -> ki ko u", ki=128))
            xt.append(t)
        wo_sb = wpool.tile([128, 8, D], BF16, tag="wo")
        nc.sync.dma_start(wo_sb[:, 0:4, :], wo_d[:, 0:4, :])
        nc.sync.dma_start(wo_sb[:, 4:8, :], wo_d[:, 4:8, :])

        # VscrT tiles: [p=(m%2,dk), b, q, hl, s]
        vscr = [vsp.tile([128, 2, 2, 2, 64], BF16, tag=f"v{k}",
                         name=f"vscr{k}") for k in range(8)]

        # GPSIMD cannot read PSUM; only DVE and Act can evacuate it
        copy_engines = [nc.vector.tensor_copy,
                        nc.scalar.copy]
        cp_i = 0

        def shuffle(a, psv):
            """Evacuate v-proj psum tiles of N-group a into VscrT (bf16)."""
            nonlocal cp_i
            for mrel in range(4):
                m = 4 * a + mrel
                k, m2 = m // 2, m % 2
                for ct in range(2):
                    for hl in range(2):
                        src = psv[ct][64 * hl:64 * hl + 64,
                                      mrel * 128:(mrel + 1) * 128] \
                            .rearrange("p (b s) -> p b s", b=2)
                        dst = vscr[k][64 * m2:64 * m2 + 64, :, ct, hl, :]
                        copy_engines[cp_i % 2](dst, src)
                        cp_i += 1

        def pad(ps, n):
            """Dependency-free N=64 matmuls that hold the PE's top p-state
            across DMA-arrival jitter. Target psum whose contents are dead
            (wiped by the next start=True group, or already evacuated)."""
            for _ in range(n):
                nc.tensor.matmul(ps[:, :64], wa[:], wb[:, :64], start=True,
                                 stop=True, skip_group_check=True)

        def vproj(a, psv=None, npad=0, pad_tile=None, midpad=0):
            if psv is None:
                psv = [ps_v.tile([128, 512], F32, tag="proj",
                                 name=f"psv{a}_{ct}") for ct in range(2)]
            if pad_tile is None:
                pad_tile = psv[0]
            pad(pad_tile, npad)
            if a == 0:
                # two half-N groups so the two small xt0 DMA pieces are
                # consumed as they land
                for h in range(2):
                    if h == 1 and midpad:
                        pad(pad_tile, midpad)
                    for ct in range(2):
                        for k in range(8):
                            nc.tensor.matmul(
                                psv[ct][:, h * 256:(h + 1) * 256],
                                wv_ap(k, ct), xt0[h][:, k, :],
                                start=(k == 0), stop=(k == 7),
                                skip_group_check=True)
            else:
                for ct in range(2):
                    for k in range(8):
                        nc.tensor.matmul(psv[ct][:], wv_ap(k, ct),
                                         xt[a][:, k, :],
                                         start=(k == 0), stop=(k == 7),
                                         skip_group_check=True)
            return psv

        # wave-1 out-proj (batch 0): 4 psum tiles accumulated k-major so the
        # PE can chase the shuffle pipeline
        po1 = None

        def w1_tiles():
            nonlocal po1
            po1 = [ps_o.tile([128, 512], F32, tag="wo",
                             name=f"po0_{q}_{n}")
                   for q in range(2) for n in range(2)]

        def w1_k(k):
            for q in range(2):
                for n in range(2):
                    nc.tensor.matmul(
                        po1[2 * q + n][:], vscr[k][:, 0, q, :, :],
                        wo_sb[:, k, n * 512:(n + 1) * 512],
                        start=(k == 0), stop=(k == 7),
                        skip_group_check=True)

        def evac(ps, b, q, n, eng):
            ob = outp.tile([128, 512], BF16, tag="ob", name=f"ob{b}_{q}_{n}")
            eng(ob[:], ps[:])
            ring = nc.sync if n == 0 else nc.scalar
            ring.dma_start(out_d[b, q, :, n * 512:(n + 1) * 512], ob[:])

        # ---- pipeline ----
        # v-proj paced by the input stream; each group's shuffle trails it
        psv1 = [ps_v.tile([128, 512], F32, tag="proj",
                          name=f"psv1_{ct}") for ct in range(2)]
        psv0 = vproj(0, psv0, pad_tile=psv1[0], midpad=4)
        vproj(1, psv1, npad=8)
        shuffle(0, psv0)
        psv2 = vproj(2, npad=4)
        shuffle(1, psv1)
        psv3 = vproj(3, npad=4)
        shuffle(2, psv2)
        pad(psv2[0], 4)
        shuffle(3, psv3)
        # wave-1 out-proj (batch 0): k0-3 k-major (woT first half lands
        # just ahead), then per-tile k4-7 + evacuation so outputs pipeline
        w1_tiles()
        for k in range(4):
            w1_k(k)
        for q in range(2):
            for n in range(2):
                po = po1[2 * q + n]
                for k in range(4, 8):
                    nc.tensor.matmul(po[:], vscr[k][:, 0, q, :, :],
                                     wo_sb[:, k, n * 512:(n + 1) * 512],
                                     start=False, stop=(k == 7),
                                     skip_group_check=True)
                evac(po, 0, q, n,
                     nc.vector.tensor_copy if n == 0 else nc.scalar.copy)
        # wave-2 (batch 1): tile-major so evacuation pipelines. The final
        # 512 columns run as two quarter-width tiles so the very last
        # matmul->copy->DMA chain is half-size (matmul cost is
        # column-limited, so the extra instructions are free; the 5th psum
        # tile reuses a slot freed by wave-2's first evacuation).
        for q in range(2):
            for n in range(2):
                if (q, n) != (1, 1):
                    po = ps_o.tile([128, 512], F32, tag="wo",
                                   name=f"po1_{q}_{n}")
                    for k in range(8):
                        nc.tensor.matmul(po[:], vscr[k][:, 1, q, :, :],
                                         wo_sb[:, k, n * 512:(n + 1) * 512],
                                         start=(k == 0), stop=(k == 7))
                    evac(po, 1, q, n,
                         nc.vector.tensor_copy if n == 0 else nc.scalar.copy)
        for half in range(2):
            po = ps_o.tile([128, 512], F32, tag="wo", name=f"po1_l{half}")
            c0 = 512 + half * 256
            for k in range(8):
                nc.tensor.matmul(po[:, 0:256], vscr[k][:, 1, 1, :, :],
                                 wo_sb[:, k, c0:c0 + 256],
                                 start=(k == 0), stop=(k == 7),
                                 skip_group_check=True)
            ob = outp.tile([128, 256], BF16, tag=f"obl{half}",
                           name=f"ob_l{half}")
            eng = nc.vector.tensor_copy if half == 0 else nc.scalar.copy
            eng(ob[:], po[:, 0:256])
            ring = nc.sync if half == 0 else nc.scalar
            ring.dma_start(out_d[1, 1, :, c0:c0 + 256], ob[:])


def _get_module():
    global _CACHED
    if _CACHED is None:
        _CACHED = _build_module()
    return _CACHED


def kernel(x, mask, Wq, Wk, Wv, Wo):
    global LAST_RESULTS
    x = np.asarray(x, dtype=np.float32)
    Wv = np.asarray(Wv, dtype=np.float32)
    Wo = np.asarray(Wo, dtype=np.float32)

    b, t, d = x.shape
    assert (b, t, d) == (B, T, D), (b, t, d)

    # x^T slabs per s-group, token columns permuted to u = m*128 + b*64 + s_lo
    # (original t = 16*(64g + s_lo) + m)
    xTs = []
    for g in range(SG):
        xs = x[:, 1024 * g:1024 * (g + 1), :]          # [b, 1024, d]
        xs = xs.reshape(B, 64, 16, D).transpose(3, 2, 0, 1)  # [d, m, b, s]
        xTs.append(np.ascontiguousarray(xs.reshape(D, TOK)).astype(BF16_NP))

    # woT[p=(m%2,dk), k, n] = Wo.T[64*(2k+m%2)+dk, n]
    woT = Wo.T.reshape(8, 2, 64, D).transpose(1, 2, 0, 3)
    woT = np.ascontiguousarray(woT.reshape(128, 8, D)).astype(BF16_NP)

    in_maps = []
    for c in range(NCORES):
        j, g = c // SG, c % SG
        wv_c = Wv[:, 256 * j:256 * j + 256]
        wv_c = np.ascontiguousarray(
            wv_c.reshape(8, 128, 2, 128).transpose(1, 0, 2, 3)
        ).astype(BF16_NP)
        in_maps.append({"xT": xTs[g], "woT": woT, "wv": wv_c})

    nc = _get_module()
    res = run_bass_kernel_spmd(nc, in_maps, list(range(NCORES)))
    LAST_RESULTS = res

    out = np.empty((B, T, D), dtype=np.float32)
    F = out.reshape(B, 16, 2, 64, D)        # (b, h, g, s_lo, n)
    for c in range(NCORES):
        j, g = c // SG, c % SG
        ob = np.asarray(res.results[c]["out"]).astype(np.float32)
        F[:, 4 * j:4 * j + 4, g] = ob.reshape(B, 4, 64, D)
    return out
